# revision 19
# baseline (speedup 1.0000x reference)
"""Trainium2 Bass kernel for nn_AttentionLayer (B=2, S=2048, D=1024, H=16, dh=64).

Sharding: head-parallel across 8 NeuronCores — each core computes the Q/K/V
projections for its 2 heads (column slices of Wq/Wk/Wv), causal attention for
its 4 (batch, head) pairs, then an AllToAll exchanges per-head context so each
core runs the output projection for 1/8 of the tokens.

All matmuls run in float32r (tf32-class PE mode, ~4x fp32 throughput,
rel err ~1e-4). Softmax skips the max-subtraction (|scores| <= 8 after the
1/sqrt(64) scale, since q/k are tanh outputs), so exp is a single ACT pass and
row sums come from an appended ones-column in the alpha @ V matmul.

The AllToAll is split per local head: the h=0 exchange overlaps the h=1
attention compute, and the output projection accumulates each 64-row half as
soon as its exchange lands (K=64 row-packed matmuls).

Self-contained: accepts the full unsharded inputs, returns the full output.
"""

import os

import numpy as np

import concourse.bass as bass
import concourse.mybir as mybir
import concourse.tile as tile
from concourse import bacc
from concourse.bass_utils import run_bass_kernel_spmd

B, S, D = 2, 2048, 1024
H, DH = 16, 64
N_CORES = 8
HPC = H // N_CORES          # heads per core (2)
LC = HPC * DH               # local projection columns (128)
T = B * S                   # total tokens (4096)
TBLK = T // N_CORES         # tokens per output block (512)
NEG = -1.0e9

f32 = mybir.dt.float32
f32r = mybir.dt.float32r

SINGLE_A2A = bool(int(os.environ.get("K_SINGLE_A2A", "0")))
NO_SCALAR_DMA = bool(int(os.environ.get("K_NO_SCALAR_DMA", "0")))

_CACHE = {}
LAST_RESULTS = None


def _build():
    nc = bacc.Bacc("TRN2", target_bir_lowering=False, debug=False,
                   num_devices=N_CORES)

    statesT = nc.dram_tensor("statesT", [D, T], f32r, kind="ExternalInput")
    wq = nc.dram_tensor("wq", [D, LC], f32r, kind="ExternalInput")
    wk = nc.dram_tensor("wk", [D, LC], f32r, kind="ExternalInput")
    wv = nc.dram_tensor("wv", [D, LC], f32r, kind="ExternalInput")
    wo = nc.dram_tensor("wo", [D, D], f32r, kind="ExternalInput")
    bq = nc.dram_tensor("bq", [LC, 1], f32, kind="ExternalInput")
    bk = nc.dram_tensor("bk", [LC, 1], f32, kind="ExternalInput")
    bv = nc.dram_tensor("bv", [LC, 1], f32, kind="ExternalInput")
    bo = nc.dram_tensor("bo", [D, 1], f32, kind="ExternalInput")
    masks = nc.dram_tensor("masks", [128, 4, 512], f32, kind="ExternalInput")
    ident = nc.dram_tensor("ident", [128, 128], f32r, kind="ExternalInput")
    ones = nc.dram_tensor("ones", [128, 64], f32r, kind="ExternalInput")

    if SINGLE_A2A:
        a2a_in_c = nc.dram_tensor("a2a_in_c", [N_CORES, LC, TBLK], f32r)
        a2a_out_c = nc.dram_tensor("a2a_out_c", [N_CORES, LC, TBLK], f32r)
    else:
        a2a_in = [nc.dram_tensor(f"a2a_in{h}", [N_CORES, DH, TBLK], f32r)
                  for h in range(HPC)]
        a2a_out = [nc.dram_tensor(f"a2a_out{h}", [N_CORES, DH, TBLK], f32r)
                   for h in range(HPC)]
    out = nc.dram_tensor("out", [D, TBLK], f32, kind="ExternalOutput")

    Tanh = mybir.ActivationFunctionType.Tanh
    Exp = mybir.ActivationFunctionType.Exp

    with tile.TileContext(nc) as tc:
        with (
            tc.tile_pool(name="consts", bufs=1) as consts,
            tc.tile_pool(name="persist", bufs=1) as persist,
            tc.tile_pool(name="stream", bufs=4) as stream,
            tc.tile_pool(name="vtp", bufs=2) as vtp,
            tc.tile_pool(name="etp", bufs=5) as etp,
            tc.tile_pool(name="cxp", bufs=2) as cxp,
            tc.tile_pool(name="outp", bufs=3) as outp,
            # one PSUM pool for the whole program: tag "mm" [128,2,512] x3 =
            # 6 banks, tag "acc" [128,512] x2 = 2 banks -> 8 banks total
            tc.tile_pool(name="psum", bufs=1, space="PSUM") as psum,
        ):
            # ---- constants / weights in SBUF ----
            wq_sb = consts.tile([128, 8, LC], f32r)
            wk_sb = consts.tile([128, 8, LC], f32r)
            wv_sb = consts.tile([128, 8, LC], f32r)
            nc.sync.dma_start(out=wq_sb, in_=wq.ap().rearrange("(k p) l -> p k l", p=128))
            nc.sync.dma_start(out=wk_sb, in_=wk.ap().rearrange("(k p) l -> p k l", p=128))
            nc.sync.dma_start(out=wv_sb, in_=wv.ap().rearrange("(k p) l -> p k l", p=128))
            bq_sb = consts.tile([LC, 1], f32)
            bk_sb = consts.tile([LC, 1], f32)
            bv_sb = consts.tile([LC, 1], f32)
            nc.sync.dma_start(out=bq_sb, in_=bq[:, :])
            nc.sync.dma_start(out=bk_sb, in_=bk[:, :])
            nc.sync.dma_start(out=bv_sb, in_=bv[:, :])
            masks_sb = consts.tile([128, 4, 512], f32)
            nc.scalar.dma_start(out=masks_sb, in_=masks[:, :, :])
            ident_sb = consts.tile([128, 128], f32r)
            nc.scalar.dma_start(out=ident_sb, in_=ident[:, :])
            ones_sb = consts.tile([128, 64], f32r)
            nc.sync.dma_start(out=ones_sb, in_=ones[:, :])
            # wo/bo are only needed by phase 3; keep them on the scalar
            # (ACT) HWDGE queue behind the statesT tiles it also carries.
            wo_sb = persist.tile([128, 8, D], f32r)
            bo_sb = consts.tile([128, 8, 1], f32)

            # ---- phase 1: Q/K/V projections (transposed layout) ----
            qt_sb = persist.tile([128, T], f32r)
            kt_sb = persist.tile([128, T], f32r)
            # v5: per 128-token tile, [tok_local, (h0 V | ones | h1 V | ones)]
            v5_sb = persist.tile([128, T // 128, 2 * (DH + 1)], f32r)
            nc.vector.tensor_copy(
                v5_sb.rearrange("p t (a b) -> p (t a) b", a=2)[:, :, DH:DH + 1].opt(),
                ones_sb[:, :].opt(),
            )

            for tt in range(T // 1024):  # 4 double-width token tiles
                acc_q = psum.tile([128, 2, 512], f32, tag="mm", bufs=3)
                acc_k = psum.tile([128, 2, 512], f32, tag="mm", bufs=3)
                acc_v = psum.tile([128, 2, 512], f32, tag="mm", bufs=3)
                for kk in range(8):
                    st = stream.tile([128, 1024], f32r, tag="st")
                    dma_eng = nc.sync if (NO_SCALAR_DMA or kk % 2 == 0) else nc.scalar
                    dma_eng.dma_start(
                        out=st,
                        in_=statesT[128 * kk:128 * (kk + 1),
                                    1024 * tt:1024 * (tt + 1)],
                    )
                    for acc, w_sb in ((acc_q, wq_sb), (acc_k, wk_sb), (acc_v, wv_sb)):
                        for half in range(2):
                            nc.tensor.matmul(acc[:, half, :], w_sb[:, kk, :],
                                             st[:, 512 * half:512 * (half + 1)],
                                             start=(kk == 0), stop=(kk == 7))
                sl = slice(1024 * tt, 1024 * (tt + 1))
                nc.scalar.activation(out=qt_sb[:, sl], in_=acc_q, func=Tanh, bias=bq_sb)
                nc.scalar.activation(out=kt_sb[:, sl], in_=acc_k, func=Tanh, bias=bk_sb)
                vt_c = vtp.tile([128, 1024], f32r, tag="vt")
                nc.scalar.activation(out=vt_c, in_=acc_v, func=Tanh, bias=bv_sb)
                # transpose each 128-col block of vt into v5 (both heads at once)
                for j in range(8):
                    t_idx = 8 * tt + j
                    trp = psum.tile([128, 512], f32r, tag="acc", bufs=2)
                    nc.tensor.transpose(trp[:, 0:128],
                                        vt_c[:, 128 * j:128 * (j + 1)], ident_sb)
                    nc.scalar.copy(
                        v5_sb.rearrange("p t (a b) -> p t a b", a=2)[:, t_idx, :, 0:DH],
                        trp[:, 0:128].rearrange("p (a b) -> p a b", a=2),
                    )

            # ---- phase 2: causal attention, h-outer for split A2A ----
            for h in range(HPC):
                p0 = DH * h
                for qi in range(4):
                    for b in range(B):
                        nkt = 4 * qi + 4       # causal kt tiles (128 wide)
                        q_lo = 2048 * b + 512 * qi
                        ets = []
                        for ch in range(nkt // 2):
                            stp = psum.tile([128, 2, 512], f32, tag="mm", bufs=3)
                            for j in range(2):
                                ktj = 2 * ch + j
                                k_lo = 2048 * b + 128 * ktj
                                nc.tensor.matmul(
                                    stp[:, j, :],
                                    kt_sb[p0:p0 + DH, k_lo:k_lo + 128],
                                    qt_sb[p0:p0 + DH, q_lo:q_lo + 512],
                                    start=True, stop=True,
                                )
                            if ch >= 2 * qi:  # diagonal chunk -> causal bias
                                moff = (ch - 2 * qi) * 2
                                nc.vector.tensor_add(stp, stp,
                                                     masks_sb[:, moff:moff + 2, :])
                            et = etp.tile([128, 2, 512], f32r, tag="et")
                            nc.scalar.activation(out=et, in_=stp, func=Exp,
                                                 scale=0.125)
                            ets.append(et)
                        # ctx^T (+ row sums via ones column): [65, 512]
                        ctxp = psum.tile([128, 512], f32, tag="acc", bufs=2)
                        for ch in range(nkt // 2):
                            for j in range(2):
                                ktj = 2 * ch + j
                                t_idx = 16 * b + ktj
                                nc.tensor.matmul(
                                    ctxp[0:DH + 1, :],
                                    v5_sb[:, t_idx, 65 * h:65 * h + 65],
                                    ets[ch][:, j, :],
                                    start=(ktj == 0), stop=(ktj == nkt - 1),
                                )
                        # copy ctx+l out of PSUM eagerly, then normalize
                        # from SBUF so the PSUM slot recycles fast
                        cl_sb = cxp.tile([DH + 1, 512], f32, tag="cl")
                        nc.vector.tensor_copy(cl_sb, ctxp[0:DH + 1, :])
                        r_sb = cxp.tile([128, 512], f32r, tag="r")
                        with nc.allow_low_precision(reason="f32r == f32 storage"):
                            nc.vector.reciprocal(out=r_sb[DH:DH + 1, :],
                                                 in_=cl_sb[DH:DH + 1, :])
                        rbp = psum.tile([128, 512], f32, tag="acc", bufs=2)
                        nc.tensor.matmul(rbp[0:DH, :], ones_sb[DH:DH + 1, :],
                                         r_sb[DH:DH + 1, :], start=True, stop=True)
                        rb_sb = cxp.tile([DH, 512], f32, tag="rb")
                        nc.vector.tensor_copy(rb_sb, rbp[0:DH, :])
                        cx = cxp.tile([DH, 512], f32r, tag="cx")
                        nc.vector.tensor_mul(cx, cl_sb[0:DH, :], rb_sb)
                        tb_idx = 4 * b + qi
                        if SINGLE_A2A:
                            nc.sync.dma_start(
                                out=a2a_in_c[tb_idx, p0:p0 + DH, :], in_=cx)
                        else:
                            nc.sync.dma_start(out=a2a_in[h][tb_idx, :, :], in_=cx)
                if not SINGLE_A2A:
                    # per-head exchange: h=0 overlaps h=1 compute
                    nc.gpsimd.collective_compute(
                        "AllToAll", mybir.AluOpType.bypass,
                        replica_groups=[list(range(N_CORES))],
                        ins=[a2a_in[h][:].opt()], outs=[a2a_out[h][:].opt()],
                    )
            if SINGLE_A2A:
                nc.gpsimd.collective_compute(
                    "AllToAll", mybir.AluOpType.bypass,
                    replica_groups=[list(range(N_CORES))],
                    ins=[a2a_in_c[:].opt()], outs=[a2a_out_c[:].opt()],
                )

            # ---- phase 3: output projection, per-half accumulation ----
            wo_eng = nc.sync if NO_SCALAR_DMA else nc.scalar
            wo_eng.dma_start(out=wo_sb,
                             in_=wo.ap().rearrange("(k p) o -> p k o", p=128))
            wo_eng.dma_start(out=bo_sb,
                             in_=bo.ap().rearrange("(k p) one -> p k one", p=128))
            cxt0s, cxt1s = [], []
            for kc in range(8):
                cxt0 = outp.tile([128, 512], f32r, tag="cxt0", bufs=8)
                cxt1 = outp.tile([128, 512], f32r, tag="cxt1", bufs=8)
                if SINGLE_A2A:
                    nc.sync.dma_start(out=cxt0[0:DH, :], in_=a2a_out_c[kc, 0:DH, :])
                    nc.scalar.dma_start(out=cxt1[DH:128, :], in_=a2a_out_c[kc, DH:128, :])
                else:
                    nc.sync.dma_start(out=cxt0[0:DH, :], in_=a2a_out[0][kc, :, :])
                    nc.scalar.dma_start(out=cxt1[DH:128, :], in_=a2a_out[1][kc, :, :])
                cxt0s.append(cxt0)
                cxt1s.append(cxt1)
            for oc in range(8):
                # separate PSUM banks per K=64 half: the two row groups run
                # concurrently on the PE and may not share a bank
                op0 = psum.tile([128, 512], f32, tag="acc", bufs=2)
                op1 = psum.tile([128, 512], f32, tag="acc", bufs=2)
                osl = slice(128 * oc, 128 * (oc + 1))
                for kc in range(8):
                    nc.tensor.matmul(op0, wo_sb[0:DH, kc, osl], cxt0s[kc][0:DH, :],
                                     start=(kc == 0), stop=(kc == 7))
                for kc in range(8):
                    nc.tensor.matmul(op1, wo_sb[DH:128, kc, osl],
                                     cxt1s[kc][DH:128, :],
                                     start=(kc == 0), stop=(kc == 7))
                s1 = outp.tile([128, 512], f32, tag="s1", bufs=2)
                nc.vector.tensor_copy(s1, op0)
                nc.vector.tensor_add(s1, s1, op1)
                osb = outp.tile([128, 512], f32, tag="osb", bufs=2)
                nc.scalar.activation(out=osb, in_=s1, func=Tanh, bias=bo_sb[:, oc, :])
                nc.sync.dma_start(out=out[osl, :], in_=osb)

    nc.compile()
    return nc


def _get_nc():
    if "nc" not in _CACHE:
        _CACHE["nc"] = _build()
    return _CACHE["nc"]


def _make_masks():
    kt_local = np.arange(128)[:, None, None]
    j = np.arange(4)[None, :, None]
    q_local = np.arange(512)[None, None, :]
    return np.where(q_local >= 128 * j + kt_local, 0.0, NEG).astype(np.float32)


def kernel(states, Wq, bq, Wk, bk, Wv, bv, Wo, bo):
    global LAST_RESULTS
    states = np.asarray(states, dtype=np.float32)
    Wq, Wk, Wv, Wo = (np.asarray(w, dtype=np.float32) for w in (Wq, Wk, Wv, Wo))
    bq, bk, bv, bo = (np.asarray(x, dtype=np.float32) for x in (bq, bk, bv, bo))

    statesT = np.ascontiguousarray(states.reshape(T, D).T)
    masks = _make_masks()
    ident = np.eye(128, dtype=np.float32)
    ones = np.ones((128, 64), dtype=np.float32)

    in_maps = []
    for c in range(N_CORES):
        sl = slice(LC * c, LC * (c + 1))
        in_maps.append({
            "statesT": statesT,
            "wq": np.ascontiguousarray(Wq[:, sl]),
            "wk": np.ascontiguousarray(Wk[:, sl]),
            "wv": np.ascontiguousarray(Wv[:, sl]),
            "wo": Wo,
            "bq": np.ascontiguousarray(bq[sl]).reshape(LC, 1),
            "bk": np.ascontiguousarray(bk[sl]).reshape(LC, 1),
            "bv": np.ascontiguousarray(bv[sl]).reshape(LC, 1),
            "bo": bo.reshape(D, 1),
            "masks": masks,
            "ident": ident,
            "ones": ones,
        })

    nc = _get_nc()
    res = run_bass_kernel_spmd(nc, in_maps, core_ids=list(range(N_CORES)))
    LAST_RESULTS = res

    full = np.empty((T, D), dtype=np.float32)
    for c in range(N_CORES):
        full[TBLK * c:TBLK * (c + 1), :] = res.results[c]["out"].T
    return full.reshape(B, S, D)


# revision 21
# speedup vs baseline: 1.1395x; 1.1395x over previous
"""Trainium2 Bass kernel for nn_AttentionLayer (B=2, S=2048, D=1024, H=16, dh=64).

Sharding: head-parallel across 8 NeuronCores — each core computes the Q/K/V
projections for its 2 heads (column slices of Wq/Wk/Wv), causal attention for
its 4 (batch, head) pairs, then an AllToAll exchanges per-head context so each
core runs the output projection for 1/8 of the tokens.

All matmuls run in float32r (tf32-class PE mode, ~4x fp32 throughput,
rel err ~1e-4). Softmax skips the max-subtraction (|scores| <= 8 after the
1/sqrt(64) scale, since q/k are tanh outputs), so exp is a single ACT pass and
row sums come from an appended ones-column in the alpha @ V matmul.

The AllToAll is split per local head: the h=0 exchange overlaps the h=1
attention compute, and the output projection accumulates each 64-row half as
soon as its exchange lands (K=64 row-packed matmuls).

Self-contained: accepts the full unsharded inputs, returns the full output.
"""

import os

import numpy as np

import concourse.bass as bass
import concourse.mybir as mybir
import concourse.tile as tile
from concourse import bacc
from concourse.bass_utils import run_bass_kernel_spmd

B, S, D = 2, 2048, 1024
H, DH = 16, 64
N_CORES = 8
HPC = H // N_CORES          # heads per core (2)
LC = HPC * DH               # local projection columns (128)
T = B * S                   # total tokens (4096)
TBLK = T // N_CORES         # tokens per output block (512)
NEG = -1.0e9

f32 = mybir.dt.float32
f32r = mybir.dt.float32r

SINGLE_A2A = bool(int(os.environ.get("K_SINGLE_A2A", "0")))
NO_SCALAR_DMA = bool(int(os.environ.get("K_NO_SCALAR_DMA", "0")))

_CACHE = {}
LAST_RESULTS = None


def _build():
    nc = bacc.Bacc("TRN2", target_bir_lowering=False, debug=False,
                   num_devices=N_CORES)

    statesT = nc.dram_tensor("statesT", [D, T], f32r, kind="ExternalInput")
    wq = nc.dram_tensor("wq", [D, LC], f32r, kind="ExternalInput")
    wk = nc.dram_tensor("wk", [D, LC], f32r, kind="ExternalInput")
    wv = nc.dram_tensor("wv", [D, LC], f32r, kind="ExternalInput")
    wo = nc.dram_tensor("wo", [D, D], f32r, kind="ExternalInput")
    bq = nc.dram_tensor("bq", [LC, 1], f32, kind="ExternalInput")
    bk = nc.dram_tensor("bk", [LC, 1], f32, kind="ExternalInput")
    bv = nc.dram_tensor("bv", [LC, 1], f32, kind="ExternalInput")
    bo = nc.dram_tensor("bo", [D, 1], f32, kind="ExternalInput")
    masks = nc.dram_tensor("masks", [128, 4, 512], mybir.dt.bfloat16, kind="ExternalInput")
    ident = nc.dram_tensor("ident", [128, 128], f32r, kind="ExternalInput")
    ones = nc.dram_tensor("ones", [128, 64], f32r, kind="ExternalInput")

    if SINGLE_A2A:
        a2a_in_c = nc.dram_tensor("a2a_in_c", [N_CORES, LC, TBLK], f32r)
        a2a_out_c = nc.dram_tensor("a2a_out_c", [N_CORES, LC, TBLK], f32r)
    else:
        a2a_in = [nc.dram_tensor(f"a2a_in{h}", [N_CORES, DH, TBLK], f32r)
                  for h in range(HPC)]
        a2a_out = [nc.dram_tensor(f"a2a_out{h}", [N_CORES, DH, TBLK], f32r)
                   for h in range(HPC)]
    out = nc.dram_tensor("out", [D, TBLK], f32, kind="ExternalOutput")

    Tanh = mybir.ActivationFunctionType.Tanh
    Exp = mybir.ActivationFunctionType.Exp

    with tile.TileContext(nc) as tc:
        with (
            tc.tile_pool(name="consts", bufs=1) as consts,
            tc.tile_pool(name="persist", bufs=1) as persist,
            tc.tile_pool(name="stream", bufs=3) as stream,
            tc.tile_pool(name="vtp", bufs=2) as vtp,
            tc.tile_pool(name="etp", bufs=4) as etp,
            tc.tile_pool(name="cxp", bufs=2) as cxp,
            tc.tile_pool(name="outp", bufs=3) as outp,
            # one PSUM pool for the whole program: tag "mm" [128,2,512] x3 =
            # 6 banks, tag "acc" [128,512] x2 = 2 banks -> 8 banks total
            tc.tile_pool(name="psum", bufs=1, space="PSUM") as psum,
        ):
            # ---- constants / weights in SBUF ----
            wq_sb = consts.tile([128, 8, LC], f32r)
            wk_sb = consts.tile([128, 8, LC], f32r)
            wv_sb = consts.tile([128, 8, LC], f32r)
            nc.sync.dma_start(out=wq_sb, in_=wq.ap().rearrange("(k p) l -> p k l", p=128))
            nc.sync.dma_start(out=wk_sb, in_=wk.ap().rearrange("(k p) l -> p k l", p=128))
            nc.sync.dma_start(out=wv_sb, in_=wv.ap().rearrange("(k p) l -> p k l", p=128))
            bq_sb = consts.tile([LC, 1], f32)
            bk_sb = consts.tile([LC, 1], f32)
            bv_sb = consts.tile([LC, 1], f32)
            nc.sync.dma_start(out=bq_sb, in_=bq[:, :])
            nc.sync.dma_start(out=bk_sb, in_=bk[:, :])
            nc.sync.dma_start(out=bv_sb, in_=bv[:, :])
            masks_sb = consts.tile([128, 4, 512], mybir.dt.bfloat16)
            nc.scalar.dma_start(out=masks_sb, in_=masks[:, :, :])
            ident_sb = consts.tile([128, 128], f32r)
            nc.scalar.dma_start(out=ident_sb, in_=ident[:, :])
            ones_sb = consts.tile([128, 64], f32r)
            nc.sync.dma_start(out=ones_sb, in_=ones[:, :])
            # wo/bo are only needed by phase 3; keep them on the scalar
            # (ACT) HWDGE queue behind the statesT tiles it also carries.
            wo_sb = persist.tile([128, 8, D], f32r)
            bo_sb = consts.tile([128, 8, 1], f32)

            # ---- phase 1: Q/K/V projections (transposed layout) ----
            qt_sb = persist.tile([128, T], f32r)
            kt_sb = persist.tile([128, T], f32r)
            # v5: per 128-token tile, [tok_local, (h0 V | ones | h1 V | ones)]
            v5_sb = persist.tile([128, T // 128, 2 * (DH + 1)], f32r)
            nc.vector.tensor_copy(
                v5_sb.rearrange("p t (a b) -> p (t a) b", a=2)[:, :, DH:DH + 1].opt(),
                ones_sb[:, :].opt(),
            )

            for tt in range(T // 1024):  # 4 double-width token tiles
                acc_q = psum.tile([128, 2, 512], f32, tag="mm", bufs=3)
                acc_k = psum.tile([128, 2, 512], f32, tag="mm", bufs=3)
                acc_v = psum.tile([128, 2, 512], f32, tag="mm", bufs=3)
                for kk in range(8):
                    st = stream.tile([128, 1024], f32r, tag="st")
                    dma_eng = nc.sync if (NO_SCALAR_DMA or kk % 2 == 0) else nc.scalar
                    dma_eng.dma_start(
                        out=st,
                        in_=statesT[128 * kk:128 * (kk + 1),
                                    1024 * tt:1024 * (tt + 1)],
                    )
                    for acc, w_sb in ((acc_q, wq_sb), (acc_k, wk_sb), (acc_v, wv_sb)):
                        for half in range(2):
                            nc.tensor.matmul(acc[:, half, :], w_sb[:, kk, :],
                                             st[:, 512 * half:512 * (half + 1)],
                                             start=(kk == 0), stop=(kk == 7))
                sl = slice(1024 * tt, 1024 * (tt + 1))
                nc.scalar.activation(out=qt_sb[:, sl], in_=acc_q, func=Tanh, bias=bq_sb)
                nc.scalar.activation(out=kt_sb[:, sl], in_=acc_k, func=Tanh, bias=bk_sb)
                vt_c = vtp.tile([128, 1024], f32r, tag="vt")
                nc.scalar.activation(out=vt_c, in_=acc_v, func=Tanh, bias=bv_sb)
                # transpose each 128-col block of vt into v5 (both heads at once)
                for j in range(8):
                    t_idx = 8 * tt + j
                    trp = psum.tile([128, 512], f32r, tag="acc", bufs=2)
                    nc.tensor.transpose(trp[:, 0:128],
                                        vt_c[:, 128 * j:128 * (j + 1)], ident_sb)
                    nc.scalar.copy(
                        v5_sb.rearrange("p t (a b) -> p t a b", a=2)[:, t_idx, :, 0:DH],
                        trp[:, 0:128].rearrange("p (a b) -> p a b", a=2),
                    )

            # ---- phase 2: causal attention, h-outer for split A2A ----
            for h in range(HPC):
                p0 = DH * h
                group = []  # (cl_sb, tb_idx) pending normalization
                def flush_group():
                    # batched 1/l: copy each group's l-row to a distinct
                    # 32-aligned partition, one reciprocal serves them all
                    lb = cxp.tile([128, 512], f32, tag="lb", bufs=1)
                    for i, (cl_sb, _) in enumerate(group):
                        nc.vector.tensor_copy(lb[32 * i:32 * i + 1, :],
                                              cl_sb[DH:DH + 1, :])
                    rbat = cxp.tile([128, 512], f32r, tag="rbat", bufs=1)
                    with nc.allow_low_precision(reason="f32r == f32 storage"):
                        nc.vector.reciprocal(out=rbat, in_=lb)
                    for i, (cl_sb, tb_idx) in enumerate(group):
                        rbp = psum.tile([128, 512], f32, tag="acc", bufs=2)
                        nc.tensor.matmul(rbp[0:DH, :], ones_sb[32 * i:32 * i + 1, :],
                                         rbat[32 * i:32 * i + 1, :],
                                         start=True, stop=True,
                                         tile_position=(32 * i, 0))
                        cx = cxp.tile([DH, 512], f32r, tag="cx")
                        nc.vector.tensor_mul(cx, cl_sb[0:DH, :], rbp[0:DH, :])
                        if SINGLE_A2A:
                            nc.sync.dma_start(
                                out=a2a_in_c[tb_idx, p0:p0 + DH, :], in_=cx)
                        else:
                            nc.sync.dma_start(out=a2a_in[h][tb_idx, :, :], in_=cx)
                    group.clear()

                for qi in range(4):
                    for b in range(B):
                        nkt = 4 * qi + 4       # causal kt tiles (128 wide)
                        q_lo = 2048 * b + 512 * qi
                        ets = []
                        for ch in range(nkt // 2):
                            stp = psum.tile([128, 2, 512], f32, tag="mm", bufs=3)
                            for j in range(2):
                                ktj = 2 * ch + j
                                k_lo = 2048 * b + 128 * ktj
                                nc.tensor.matmul(
                                    stp[:, j, :],
                                    kt_sb[p0:p0 + DH, k_lo:k_lo + 128],
                                    qt_sb[p0:p0 + DH, q_lo:q_lo + 512],
                                    start=True, stop=True,
                                )
                            if ch >= 2 * qi:  # diagonal chunk -> causal bias
                                moff = (ch - 2 * qi) * 2
                                nc.vector.tensor_add(stp, stp,
                                                     masks_sb[:, moff:moff + 2, :])
                            et = etp.tile([128, 2, 512], f32r, tag="et")
                            nc.scalar.activation(out=et, in_=stp, func=Exp,
                                                 scale=0.125)
                            ets.append(et)
                        # ctx^T (+ row sums via ones column): [65, 512]
                        ctxp = psum.tile([128, 512], f32, tag="acc", bufs=2)
                        for ch in range(nkt // 2):
                            for j in range(2):
                                ktj = 2 * ch + j
                                t_idx = 16 * b + ktj
                                nc.tensor.matmul(
                                    ctxp[0:DH + 1, :],
                                    v5_sb[:, t_idx, 65 * h:65 * h + 65],
                                    ets[ch][:, j, :],
                                    start=(ktj == 0), stop=(ktj == nkt - 1),
                                )
                        # copy ctx+l out of PSUM eagerly (PSUM slot recycles)
                        cl_sb = cxp.tile([DH + 1, 512], f32, tag="cl", bufs=5)
                        nc.vector.tensor_copy(cl_sb, ctxp[0:DH + 1, :])
                        group.append((cl_sb, 4 * b + qi))
                        if len(group) == 4:
                            flush_group()
                assert not group
                if not SINGLE_A2A:
                    # per-head exchange: h=0 overlaps h=1 compute
                    nc.gpsimd.collective_compute(
                        "AllToAll", mybir.AluOpType.bypass,
                        replica_groups=[list(range(N_CORES))],
                        ins=[a2a_in[h][:].opt()], outs=[a2a_out[h][:].opt()],
                    )
            if SINGLE_A2A:
                nc.gpsimd.collective_compute(
                    "AllToAll", mybir.AluOpType.bypass,
                    replica_groups=[list(range(N_CORES))],
                    ins=[a2a_in_c[:].opt()], outs=[a2a_out_c[:].opt()],
                )

            # ---- phase 3: output projection, per-half accumulation ----
            wo_eng = nc.sync if NO_SCALAR_DMA else nc.scalar
            wo_eng.dma_start(out=wo_sb,
                             in_=wo.ap().rearrange("(k p) o -> p k o", p=128))
            wo_eng.dma_start(out=bo_sb,
                             in_=bo.ap().rearrange("(k p) one -> p k one", p=128))
            cxt0s, cxt1s = [], []
            for kc in range(8):
                cxt0 = outp.tile([128, 512], f32r, tag="cxt0", bufs=8)
                cxt1 = outp.tile([128, 512], f32r, tag="cxt1", bufs=8)
                if SINGLE_A2A:
                    nc.sync.dma_start(out=cxt0[0:DH, :], in_=a2a_out_c[kc, 0:DH, :])
                    nc.scalar.dma_start(out=cxt1[DH:128, :], in_=a2a_out_c[kc, DH:128, :])
                else:
                    nc.sync.dma_start(out=cxt0[0:DH, :], in_=a2a_out[0][kc, :, :])
                    nc.scalar.dma_start(out=cxt1[DH:128, :], in_=a2a_out[1][kc, :, :])
                cxt0s.append(cxt0)
                cxt1s.append(cxt1)
            s0s = []
            for oc in range(8):  # h=0 half: runs as soon as A2A#1 lands
                op0 = psum.tile([128, 512], f32, tag="acc", bufs=2)
                osl = slice(128 * oc, 128 * (oc + 1))
                for kc in range(8):
                    nc.tensor.matmul(op0, wo_sb[0:DH, kc, osl], cxt0s[kc][0:DH, :],
                                     start=(kc == 0), stop=(kc == 7))
                s0 = outp.tile([128, 512], f32, tag="s0", bufs=8)
                nc.vector.tensor_copy(s0, op0)
                s0s.append(s0)
            for oc in range(8):  # h=1 half after A2A#2, then combine
                op1 = psum.tile([128, 512], f32, tag="acc", bufs=2)
                osl = slice(128 * oc, 128 * (oc + 1))
                for kc in range(8):
                    nc.tensor.matmul(op1, wo_sb[DH:128, kc, osl],
                                     cxt1s[kc][DH:128, :],
                                     start=(kc == 0), stop=(kc == 7))
                s1 = outp.tile([128, 512], f32, tag="s1", bufs=2)
                nc.vector.tensor_add(s1, s0s[oc], op1)
                osb = outp.tile([128, 512], f32, tag="osb", bufs=2)
                nc.scalar.activation(out=osb, in_=s1, func=Tanh, bias=bo_sb[:, oc, :])
                nc.sync.dma_start(out=out[osl, :], in_=osb)

    nc.compile()
    return nc


def _get_nc():
    if "nc" not in _CACHE:
        _CACHE["nc"] = _build()
    return _CACHE["nc"]


def _make_masks():
    kt_local = np.arange(128)[:, None, None]
    j = np.arange(4)[None, :, None]
    q_local = np.arange(512)[None, None, :]
    import ml_dtypes
    return np.where(q_local >= 128 * j + kt_local, 0.0, NEG).astype(ml_dtypes.bfloat16)


def kernel(states, Wq, bq, Wk, bk, Wv, bv, Wo, bo):
    global LAST_RESULTS
    states = np.asarray(states, dtype=np.float32)
    Wq, Wk, Wv, Wo = (np.asarray(w, dtype=np.float32) for w in (Wq, Wk, Wv, Wo))
    bq, bk, bv, bo = (np.asarray(x, dtype=np.float32) for x in (bq, bk, bv, bo))

    statesT = np.ascontiguousarray(states.reshape(T, D).T)
    masks = _make_masks()
    ident = np.eye(128, dtype=np.float32)
    ones = np.ones((128, 64), dtype=np.float32)

    in_maps = []
    for c in range(N_CORES):
        sl = slice(LC * c, LC * (c + 1))
        in_maps.append({
            "statesT": statesT,
            "wq": np.ascontiguousarray(Wq[:, sl]),
            "wk": np.ascontiguousarray(Wk[:, sl]),
            "wv": np.ascontiguousarray(Wv[:, sl]),
            "wo": Wo,
            "bq": np.ascontiguousarray(bq[sl]).reshape(LC, 1),
            "bk": np.ascontiguousarray(bk[sl]).reshape(LC, 1),
            "bv": np.ascontiguousarray(bv[sl]).reshape(LC, 1),
            "bo": bo.reshape(D, 1),
            "masks": masks,
            "ident": ident,
            "ones": ones,
        })

    nc = _get_nc()
    res = run_bass_kernel_spmd(nc, in_maps, core_ids=list(range(N_CORES)))
    LAST_RESULTS = res

    full = np.empty((T, D), dtype=np.float32)
    for c in range(N_CORES):
        full[TBLK * c:TBLK * (c + 1), :] = res.results[c]["out"].T
    return full.reshape(B, S, D)


# revision 22
# speedup vs baseline: 1.2129x; 1.0644x over previous
"""Trainium2 Bass kernel for nn_AttentionLayer (B=2, S=2048, D=1024, H=16, dh=64).

Sharding: head-parallel across 8 NeuronCores — each core computes the Q/K/V
projections for its 2 heads (column slices of Wq/Wk/Wv), causal attention for
its 4 (batch, head) pairs, then an AllToAll exchanges per-head context so each
core runs the output projection for 1/8 of the tokens.

All matmuls run in float32r (tf32-class PE mode, ~4x fp32 throughput,
rel err ~1e-4). Softmax skips the max-subtraction (|scores| <= 8 after the
1/sqrt(64) scale, since q/k are tanh outputs), so exp is a single ACT pass and
row sums come from an appended ones-column in the alpha @ V matmul.

The AllToAll is split per local head: the h=0 exchange overlaps the h=1
attention compute, and the output projection accumulates each 64-row half as
soon as its exchange lands (K=64 row-packed matmuls).

Self-contained: accepts the full unsharded inputs, returns the full output.
"""

import os

import numpy as np

import concourse.bass as bass
import concourse.mybir as mybir
import concourse.tile as tile
from concourse import bacc
from concourse.bass_utils import run_bass_kernel_spmd

B, S, D = 2, 2048, 1024
H, DH = 16, 64
N_CORES = 8
HPC = H // N_CORES          # heads per core (2)
LC = HPC * DH               # local projection columns (128)
T = B * S                   # total tokens (4096)
TBLK = T // N_CORES         # tokens per output block (512)
NEG = -1.0e9

f32 = mybir.dt.float32
f32r = mybir.dt.float32r

SINGLE_A2A = bool(int(os.environ.get("K_SINGLE_A2A", "0")))
NO_SCALAR_DMA = bool(int(os.environ.get("K_NO_SCALAR_DMA", "0")))

_CACHE = {}
LAST_RESULTS = None


def _build():
    nc = bacc.Bacc("TRN2", target_bir_lowering=False, debug=False,
                   num_devices=N_CORES)

    statesT = nc.dram_tensor("statesT", [D, T], f32r, kind="ExternalInput")
    wq = nc.dram_tensor("wq", [D, LC], f32r, kind="ExternalInput")
    wk = nc.dram_tensor("wk", [D, LC], f32r, kind="ExternalInput")
    wv = nc.dram_tensor("wv", [D, LC], f32r, kind="ExternalInput")
    wo = nc.dram_tensor("wo", [D, D], f32r, kind="ExternalInput")
    bq = nc.dram_tensor("bq", [LC, 1], f32, kind="ExternalInput")
    bk = nc.dram_tensor("bk", [LC, 1], f32, kind="ExternalInput")
    bv = nc.dram_tensor("bv", [LC, 1], f32, kind="ExternalInput")
    bo = nc.dram_tensor("bo", [D, 1], f32, kind="ExternalInput")
    masks = nc.dram_tensor("masks", [128, 4, 512], mybir.dt.bfloat16, kind="ExternalInput")
    ident = nc.dram_tensor("ident", [128, 128], f32r, kind="ExternalInput")
    ones = nc.dram_tensor("ones", [128, 64], f32r, kind="ExternalInput")

    if SINGLE_A2A:
        a2a_in_c = nc.dram_tensor("a2a_in_c", [N_CORES, LC, TBLK], f32r)
        a2a_out_c = nc.dram_tensor("a2a_out_c", [N_CORES, LC, TBLK], f32r)
    else:
        a2a_in = [nc.dram_tensor(f"a2a_in{h}", [N_CORES, DH, TBLK], f32r)
                  for h in range(HPC)]
        a2a_out = [nc.dram_tensor(f"a2a_out{h}", [N_CORES, DH, TBLK], f32r)
                   for h in range(HPC)]
    out = nc.dram_tensor("out", [D, TBLK], f32, kind="ExternalOutput")

    Tanh = mybir.ActivationFunctionType.Tanh
    Exp = mybir.ActivationFunctionType.Exp

    with tile.TileContext(nc) as tc:
        with (
            tc.tile_pool(name="consts", bufs=1) as consts,
            tc.tile_pool(name="persist", bufs=1) as persist,
            tc.tile_pool(name="stream", bufs=5) as stream,
            tc.tile_pool(name="vtp", bufs=2) as vtp,
            tc.tile_pool(name="etp", bufs=6) as etp,
            tc.tile_pool(name="cxp", bufs=2) as cxp,
            tc.tile_pool(name="outp", bufs=3) as outp,
            # one PSUM pool for the whole program: tag "mm" [128,2,512] x3 =
            # 6 banks, tag "acc" [128,512] x2 = 2 banks -> 8 banks total
            tc.tile_pool(name="psum", bufs=1, space="PSUM") as psum,
        ):
            # ---- constants / weights in SBUF ----
            wq_sb = consts.tile([128, 8, LC], f32r)
            wk_sb = consts.tile([128, 8, LC], f32r)
            wv_sb = consts.tile([128, 8, LC], f32r)
            nc.sync.dma_start(out=wq_sb, in_=wq.ap().rearrange("(k p) l -> p k l", p=128))
            nc.sync.dma_start(out=wk_sb, in_=wk.ap().rearrange("(k p) l -> p k l", p=128))
            nc.sync.dma_start(out=wv_sb, in_=wv.ap().rearrange("(k p) l -> p k l", p=128))
            bq_sb = consts.tile([LC, 1], f32)
            bk_sb = consts.tile([LC, 1], f32)
            bv_sb = consts.tile([LC, 1], f32)
            nc.sync.dma_start(out=bq_sb, in_=bq[:, :])
            nc.sync.dma_start(out=bk_sb, in_=bk[:, :])
            nc.sync.dma_start(out=bv_sb, in_=bv[:, :])
            masks_sb = consts.tile([128, 4, 512], mybir.dt.bfloat16)
            nc.scalar.dma_start(out=masks_sb, in_=masks[:, :, :])
            ident_sb = consts.tile([128, 128], f32r)
            nc.scalar.dma_start(out=ident_sb, in_=ident[:, :])
            ones_sb = consts.tile([128, 64], f32r)
            nc.sync.dma_start(out=ones_sb, in_=ones[:, :])
            # wo/bo are only needed by phase 3; keep them on the scalar
            # (ACT) HWDGE queue behind the statesT tiles it also carries.
            wo_sb = persist.tile([128, 8, D], f32r)
            bo_sb = consts.tile([128, 8, 1], f32)

            # ---- phase 1: Q/K/V projections (transposed layout) ----
            qt_sb = persist.tile([128, T], mybir.dt.bfloat16)
            kt_sb = persist.tile([128, T], mybir.dt.bfloat16)
            # v5: per 128-token tile, [tok_local, (h0 V | ones | h1 V | ones)]
            v5_sb = persist.tile([128, T // 128, 2 * (DH + 1)], f32r)
            nc.vector.tensor_copy(
                v5_sb.rearrange("p t (a b) -> p (t a) b", a=2)[:, :, DH:DH + 1].opt(),
                ones_sb[:, :].opt(),
            )

            for tt in range(T // 1024):  # 4 double-width token tiles
                acc_q = psum.tile([128, 2, 512], f32, tag="mm", bufs=3)
                acc_k = psum.tile([128, 2, 512], f32, tag="mm", bufs=3)
                acc_v = psum.tile([128, 2, 512], f32, tag="mm", bufs=3)
                for kk in range(8):
                    st = stream.tile([128, 1024], f32r, tag="st")
                    dma_eng = nc.sync if (NO_SCALAR_DMA or kk % 2 == 0) else nc.scalar
                    dma_eng.dma_start(
                        out=st,
                        in_=statesT[128 * kk:128 * (kk + 1),
                                    1024 * tt:1024 * (tt + 1)],
                    )
                    for acc, w_sb in ((acc_q, wq_sb), (acc_k, wk_sb), (acc_v, wv_sb)):
                        for half in range(2):
                            nc.tensor.matmul(acc[:, half, :], w_sb[:, kk, :],
                                             st[:, 512 * half:512 * (half + 1)],
                                             start=(kk == 0), stop=(kk == 7))
                sl = slice(1024 * tt, 1024 * (tt + 1))
                nc.scalar.activation(out=qt_sb[:, sl], in_=acc_q, func=Tanh, bias=bq_sb)
                nc.scalar.activation(out=kt_sb[:, sl], in_=acc_k, func=Tanh, bias=bk_sb)
                vt_c = vtp.tile([128, 1024], f32r, tag="vt")
                nc.scalar.activation(out=vt_c, in_=acc_v, func=Tanh, bias=bv_sb)
                # transpose each 128-col block of vt into v5 (both heads at once)
                for j in range(8):
                    t_idx = 8 * tt + j
                    trp = psum.tile([128, 512], f32r, tag="acc", bufs=2)
                    nc.tensor.transpose(trp[:, 0:128],
                                        vt_c[:, 128 * j:128 * (j + 1)], ident_sb)
                    nc.scalar.copy(
                        v5_sb.rearrange("p t (a b) -> p t a b", a=2)[:, t_idx, :, 0:DH],
                        trp[:, 0:128].rearrange("p (a b) -> p a b", a=2),
                    )

            # ---- phase 2: causal attention, h-outer for split A2A ----
            for h in range(HPC):
                p0 = DH * h
                group = []  # (cl_sb, tb_idx) pending normalization
                def flush_group():
                    # batched 1/l: copy each group's l-row to a distinct
                    # 32-aligned partition, one reciprocal serves them all
                    lb = cxp.tile([128, 512], f32, tag="lb", bufs=1)
                    for i, (cl_sb, _) in enumerate(group):
                        nc.vector.tensor_copy(lb[32 * i:32 * i + 1, :],
                                              cl_sb[DH:DH + 1, :])
                    rbat = cxp.tile([128, 512], f32r, tag="rbat", bufs=1)
                    with nc.allow_low_precision(reason="f32r == f32 storage"):
                        nc.vector.reciprocal(out=rbat, in_=lb)
                    for i, (cl_sb, tb_idx) in enumerate(group):
                        rbp = psum.tile([128, 512], f32, tag="acc", bufs=2)
                        nc.tensor.matmul(rbp[0:DH, :], ones_sb[32 * i:32 * i + 1, :],
                                         rbat[32 * i:32 * i + 1, :],
                                         start=True, stop=True,
                                         tile_position=(32 * i, 0))
                        cx = cxp.tile([DH, 512], f32r, tag="cx")
                        nc.vector.tensor_mul(cx, cl_sb[0:DH, :], rbp[0:DH, :])
                        if SINGLE_A2A:
                            nc.sync.dma_start(
                                out=a2a_in_c[tb_idx, p0:p0 + DH, :], in_=cx)
                        else:
                            nc.sync.dma_start(out=a2a_in[h][tb_idx, :, :], in_=cx)
                    group.clear()

                for qi in range(4):
                    for b in range(B):
                        nkt = 4 * qi + 4       # causal kt tiles (128 wide)
                        q_lo = 2048 * b + 512 * qi
                        ets = []
                        for ch in range(nkt // 2):
                            stp = psum.tile([128, 2, 512], f32, tag="mm", bufs=3)
                            for j in range(2):
                                ktj = 2 * ch + j
                                k_lo = 2048 * b + 128 * ktj
                                nc.tensor.matmul(
                                    stp[:, j, :],
                                    kt_sb[p0:p0 + DH, k_lo:k_lo + 128],
                                    qt_sb[p0:p0 + DH, q_lo:q_lo + 512],
                                    start=True, stop=True,
                                )
                            if ch >= 2 * qi:  # diagonal chunk -> causal bias
                                moff = (ch - 2 * qi) * 2
                                nc.vector.tensor_add(stp, stp,
                                                     masks_sb[:, moff:moff + 2, :])
                            et = etp.tile([128, 2, 512], f32r, tag="et")
                            nc.scalar.activation(out=et, in_=stp, func=Exp,
                                                 scale=0.125)
                            ets.append(et)
                        # ctx^T (+ row sums via ones column): [65, 512]
                        ctxp = psum.tile([128, 512], f32, tag="acc", bufs=2)
                        for ch in range(nkt // 2):
                            for j in range(2):
                                ktj = 2 * ch + j
                                t_idx = 16 * b + ktj
                                nc.tensor.matmul(
                                    ctxp[0:DH + 1, :],
                                    v5_sb[:, t_idx, 65 * h:65 * h + 65],
                                    ets[ch][:, j, :],
                                    start=(ktj == 0), stop=(ktj == nkt - 1),
                                )
                        # copy ctx+l out of PSUM eagerly (PSUM slot recycles)
                        cl_sb = cxp.tile([DH + 1, 512], f32, tag="cl", bufs=5)
                        nc.vector.tensor_copy(cl_sb, ctxp[0:DH + 1, :])
                        group.append((cl_sb, 4 * b + qi))
                        if len(group) == 4:
                            flush_group()
                assert not group
                if not SINGLE_A2A:
                    # per-head exchange: h=0 overlaps h=1 compute
                    nc.gpsimd.collective_compute(
                        "AllToAll", mybir.AluOpType.bypass,
                        replica_groups=[list(range(N_CORES))],
                        ins=[a2a_in[h][:].opt()], outs=[a2a_out[h][:].opt()],
                    )
            if SINGLE_A2A:
                nc.gpsimd.collective_compute(
                    "AllToAll", mybir.AluOpType.bypass,
                    replica_groups=[list(range(N_CORES))],
                    ins=[a2a_in_c[:].opt()], outs=[a2a_out_c[:].opt()],
                )

            # ---- phase 3: output projection, per-half accumulation ----
            wo_eng = nc.sync if NO_SCALAR_DMA else nc.scalar
            wo_eng.dma_start(out=wo_sb,
                             in_=wo.ap().rearrange("(k p) o -> p k o", p=128))
            wo_eng.dma_start(out=bo_sb,
                             in_=bo.ap().rearrange("(k p) one -> p k one", p=128))
            cxt0s, cxt1s = [], []
            for kc in range(8):
                cxt0 = outp.tile([128, 512], f32r, tag="cxt0", bufs=8)
                cxt1 = outp.tile([128, 512], f32r, tag="cxt1", bufs=8)
                if SINGLE_A2A:
                    nc.sync.dma_start(out=cxt0[0:DH, :], in_=a2a_out_c[kc, 0:DH, :])
                    nc.scalar.dma_start(out=cxt1[DH:128, :], in_=a2a_out_c[kc, DH:128, :])
                else:
                    nc.sync.dma_start(out=cxt0[0:DH, :], in_=a2a_out[0][kc, :, :])
                    nc.scalar.dma_start(out=cxt1[DH:128, :], in_=a2a_out[1][kc, :, :])
                cxt0s.append(cxt0)
                cxt1s.append(cxt1)
            s0s = []
            for oc in range(8):  # h=0 half: runs as soon as A2A#1 lands
                op0 = psum.tile([128, 512], f32, tag="acc", bufs=2)
                osl = slice(128 * oc, 128 * (oc + 1))
                for kc in range(8):
                    nc.tensor.matmul(op0, wo_sb[0:DH, kc, osl], cxt0s[kc][0:DH, :],
                                     start=(kc == 0), stop=(kc == 7))
                s0 = outp.tile([128, 512], f32, tag="s0", bufs=8)
                nc.vector.tensor_copy(s0, op0)
                s0s.append(s0)
            for oc in range(8):  # h=1 half after A2A#2, then combine
                op1 = psum.tile([128, 512], f32, tag="acc", bufs=2)
                osl = slice(128 * oc, 128 * (oc + 1))
                for kc in range(8):
                    nc.tensor.matmul(op1, wo_sb[DH:128, kc, osl],
                                     cxt1s[kc][DH:128, :],
                                     start=(kc == 0), stop=(kc == 7))
                s1 = outp.tile([128, 512], f32, tag="s1", bufs=2)
                nc.vector.tensor_add(s1, s0s[oc], op1)
                osb = outp.tile([128, 512], f32, tag="osb", bufs=2)
                nc.scalar.activation(out=osb, in_=s1, func=Tanh, bias=bo_sb[:, oc, :])
                nc.sync.dma_start(out=out[osl, :], in_=osb)

    nc.compile()
    return nc


def _get_nc():
    if "nc" not in _CACHE:
        _CACHE["nc"] = _build()
    return _CACHE["nc"]


def _make_masks():
    kt_local = np.arange(128)[:, None, None]
    j = np.arange(4)[None, :, None]
    q_local = np.arange(512)[None, None, :]
    import ml_dtypes
    return np.where(q_local >= 128 * j + kt_local, 0.0, NEG).astype(ml_dtypes.bfloat16)


def kernel(states, Wq, bq, Wk, bk, Wv, bv, Wo, bo):
    global LAST_RESULTS
    states = np.asarray(states, dtype=np.float32)
    Wq, Wk, Wv, Wo = (np.asarray(w, dtype=np.float32) for w in (Wq, Wk, Wv, Wo))
    bq, bk, bv, bo = (np.asarray(x, dtype=np.float32) for x in (bq, bk, bv, bo))

    statesT = np.ascontiguousarray(states.reshape(T, D).T)
    masks = _make_masks()
    ident = np.eye(128, dtype=np.float32)
    ones = np.ones((128, 64), dtype=np.float32)

    in_maps = []
    for c in range(N_CORES):
        sl = slice(LC * c, LC * (c + 1))
        in_maps.append({
            "statesT": statesT,
            "wq": np.ascontiguousarray(Wq[:, sl]),
            "wk": np.ascontiguousarray(Wk[:, sl]),
            "wv": np.ascontiguousarray(Wv[:, sl]),
            "wo": Wo,
            "bq": np.ascontiguousarray(bq[sl]).reshape(LC, 1),
            "bk": np.ascontiguousarray(bk[sl]).reshape(LC, 1),
            "bv": np.ascontiguousarray(bv[sl]).reshape(LC, 1),
            "bo": bo.reshape(D, 1),
            "masks": masks,
            "ident": ident,
            "ones": ones,
        })

    nc = _get_nc()
    res = run_bass_kernel_spmd(nc, in_maps, core_ids=list(range(N_CORES)))
    LAST_RESULTS = res

    full = np.empty((T, D), dtype=np.float32)
    for c in range(N_CORES):
        full[TBLK * c:TBLK * (c + 1), :] = res.results[c]["out"].T
    return full.reshape(B, S, D)


# revision 23
# speedup vs baseline: 1.3089x; 1.0792x over previous
"""Trainium2 Bass kernel for nn_AttentionLayer (B=2, S=2048, D=1024, H=16, dh=64).

Sharding: head-parallel across 8 NeuronCores — each core computes the Q/K/V
projections for its 2 heads (column slices of Wq/Wk/Wv), causal attention for
its 4 (batch, head) pairs, then an AllToAll exchanges per-head context so each
core runs the output projection for 1/8 of the tokens.

All matmuls run in float32r (tf32-class PE mode, ~4x fp32 throughput,
rel err ~1e-4). Softmax skips the max-subtraction (|scores| <= 8 after the
1/sqrt(64) scale, since q/k are tanh outputs), so exp is a single ACT pass and
row sums come from an appended ones-column in the alpha @ V matmul.

The AllToAll is split per local head: the h=0 exchange overlaps the h=1
attention compute, and the output projection accumulates each 64-row half as
soon as its exchange lands (K=64 row-packed matmuls).

Self-contained: accepts the full unsharded inputs, returns the full output.
"""

import os

import numpy as np

import concourse.bass as bass
import concourse.mybir as mybir
import concourse.tile as tile
from concourse import bacc
from concourse.bass_utils import run_bass_kernel_spmd

B, S, D = 2, 2048, 1024
H, DH = 16, 64
N_CORES = 8
HPC = H // N_CORES          # heads per core (2)
LC = HPC * DH               # local projection columns (128)
T = B * S                   # total tokens (4096)
TBLK = T // N_CORES         # tokens per output block (512)
NEG = -1.0e9

f32 = mybir.dt.float32
f32r = mybir.dt.float32r

SINGLE_A2A = bool(int(os.environ.get("K_SINGLE_A2A", "0")))
NO_SCALAR_DMA = bool(int(os.environ.get("K_NO_SCALAR_DMA", "0")))

_CACHE = {}
LAST_RESULTS = None


def _build():
    nc = bacc.Bacc("TRN2", target_bir_lowering=False, debug=False,
                   num_devices=N_CORES)

    statesT = nc.dram_tensor("statesT", [D, T], mybir.dt.bfloat16, kind="ExternalInput")
    wq = nc.dram_tensor("wq", [D, LC], mybir.dt.bfloat16, kind="ExternalInput")
    wk = nc.dram_tensor("wk", [D, LC], mybir.dt.bfloat16, kind="ExternalInput")
    wv = nc.dram_tensor("wv", [D, LC], mybir.dt.bfloat16, kind="ExternalInput")
    wo = nc.dram_tensor("wo", [D, D], f32r, kind="ExternalInput")
    bq = nc.dram_tensor("bq", [LC, 1], f32, kind="ExternalInput")
    bk = nc.dram_tensor("bk", [LC, 1], f32, kind="ExternalInput")
    bv = nc.dram_tensor("bv", [LC, 1], f32, kind="ExternalInput")
    bo = nc.dram_tensor("bo", [D, 1], f32, kind="ExternalInput")
    masks = nc.dram_tensor("masks", [128, 4, 512], mybir.dt.bfloat16, kind="ExternalInput")
    ident = nc.dram_tensor("ident", [128, 128], f32r, kind="ExternalInput")
    ones = nc.dram_tensor("ones", [128, 64], f32r, kind="ExternalInput")

    if SINGLE_A2A:
        a2a_in_c = nc.dram_tensor("a2a_in_c", [N_CORES, LC, TBLK], f32r)
        a2a_out_c = nc.dram_tensor("a2a_out_c", [N_CORES, LC, TBLK], f32r)
    else:
        a2a_in = [nc.dram_tensor(f"a2a_in{h}", [N_CORES, DH, TBLK], f32r)
                  for h in range(HPC)]
        a2a_out = [nc.dram_tensor(f"a2a_out{h}", [N_CORES, DH, TBLK], f32r)
                   for h in range(HPC)]
    out = nc.dram_tensor("out", [D, TBLK], f32, kind="ExternalOutput")

    Tanh = mybir.ActivationFunctionType.Tanh
    Exp = mybir.ActivationFunctionType.Exp

    with tile.TileContext(nc) as tc:
        with (
            tc.tile_pool(name="consts", bufs=1) as consts,
            tc.tile_pool(name="persist", bufs=1) as persist,
            tc.tile_pool(name="stream", bufs=5) as stream,
            tc.tile_pool(name="vtp", bufs=2) as vtp,
            tc.tile_pool(name="etp", bufs=6) as etp,
            tc.tile_pool(name="cxp", bufs=2) as cxp,
            tc.tile_pool(name="outp", bufs=3) as outp,
            # one PSUM pool for the whole program: tag "mm" [128,2,512] x3 =
            # 6 banks, tag "acc" [128,512] x2 = 2 banks -> 8 banks total
            tc.tile_pool(name="psum", bufs=1, space="PSUM") as psum,
        ):
            # ---- constants / weights in SBUF ----
            wq_sb = consts.tile([128, 8, LC], mybir.dt.bfloat16)
            wk_sb = consts.tile([128, 8, LC], mybir.dt.bfloat16)
            wv_sb = consts.tile([128, 8, LC], mybir.dt.bfloat16)
            nc.sync.dma_start(out=wq_sb, in_=wq.ap().rearrange("(k p) l -> p k l", p=128))
            nc.sync.dma_start(out=wk_sb, in_=wk.ap().rearrange("(k p) l -> p k l", p=128))
            nc.sync.dma_start(out=wv_sb, in_=wv.ap().rearrange("(k p) l -> p k l", p=128))
            bq_sb = consts.tile([LC, 1], f32)
            bk_sb = consts.tile([LC, 1], f32)
            bv_sb = consts.tile([LC, 1], f32)
            nc.sync.dma_start(out=bq_sb, in_=bq[:, :])
            nc.sync.dma_start(out=bk_sb, in_=bk[:, :])
            nc.sync.dma_start(out=bv_sb, in_=bv[:, :])
            masks_sb = consts.tile([128, 4, 512], mybir.dt.bfloat16)
            nc.scalar.dma_start(out=masks_sb, in_=masks[:, :, :])
            ident_sb = consts.tile([128, 128], f32r)
            nc.scalar.dma_start(out=ident_sb, in_=ident[:, :])
            ones_sb = consts.tile([128, 64], f32r)
            nc.sync.dma_start(out=ones_sb, in_=ones[:, :])
            # wo/bo are only needed by phase 3; keep them on the scalar
            # (ACT) HWDGE queue behind the statesT tiles it also carries.
            wo_sb = persist.tile([128, 8, D], f32r)
            bo_sb = consts.tile([128, 8, 1], f32)

            # ---- phase 1: Q/K/V projections (transposed layout) ----
            qt_sb = persist.tile([128, T], mybir.dt.bfloat16)
            kt_sb = persist.tile([128, T], mybir.dt.bfloat16)
            # v5: per 128-token tile, [tok_local, (h0 V | ones | h1 V | ones)]
            v5_sb = persist.tile([128, T // 128, 2 * (DH + 1)], f32r)
            nc.vector.tensor_copy(
                v5_sb.rearrange("p t (a b) -> p (t a) b", a=2)[:, :, DH:DH + 1].opt(),
                ones_sb[:, :].opt(),
            )

            for tt in range(T // 1024):  # 4 double-width token tiles
                acc_q = psum.tile([128, 2, 512], f32, tag="mm", bufs=3)
                acc_k = psum.tile([128, 2, 512], f32, tag="mm", bufs=3)
                acc_v = psum.tile([128, 2, 512], f32, tag="mm", bufs=3)
                for kk in range(8):
                    st = stream.tile([128, 1024], mybir.dt.bfloat16, tag="st")
                    dma_eng = nc.sync if (NO_SCALAR_DMA or kk % 2 == 0) else nc.scalar
                    dma_eng.dma_start(
                        out=st,
                        in_=statesT[128 * kk:128 * (kk + 1),
                                    1024 * tt:1024 * (tt + 1)],
                    )
                    for acc, w_sb in ((acc_q, wq_sb), (acc_k, wk_sb), (acc_v, wv_sb)):
                        for half in range(2):
                            nc.tensor.matmul(acc[:, half, :], w_sb[:, kk, :],
                                             st[:, 512 * half:512 * (half + 1)],
                                             start=(kk == 0), stop=(kk == 7))
                sl = slice(1024 * tt, 1024 * (tt + 1))
                nc.scalar.activation(out=qt_sb[:, sl], in_=acc_q, func=Tanh, bias=bq_sb)
                nc.scalar.activation(out=kt_sb[:, sl], in_=acc_k, func=Tanh, bias=bk_sb)
                vt_c = vtp.tile([128, 1024], f32r, tag="vt")
                nc.scalar.activation(out=vt_c, in_=acc_v, func=Tanh, bias=bv_sb)
                # transpose each 128-col block of vt into v5 (both heads at once)
                for j in range(8):
                    t_idx = 8 * tt + j
                    trp = psum.tile([128, 512], f32r, tag="acc", bufs=2)
                    nc.tensor.transpose(trp[:, 0:128],
                                        vt_c[:, 128 * j:128 * (j + 1)], ident_sb)
                    nc.scalar.copy(
                        v5_sb.rearrange("p t (a b) -> p t a b", a=2)[:, t_idx, :, 0:DH],
                        trp[:, 0:128].rearrange("p (a b) -> p a b", a=2),
                    )

            # ---- phase 2: causal attention, h-outer for split A2A ----
            for h in range(HPC):
                p0 = DH * h
                group = []  # (cl_sb, tb_idx) pending normalization
                def flush_group():
                    # batched 1/l: copy each group's l-row to a distinct
                    # 32-aligned partition, one reciprocal serves them all
                    lb = cxp.tile([128, 512], f32, tag="lb", bufs=1)
                    for i, (cl_sb, _) in enumerate(group):
                        nc.vector.tensor_copy(lb[32 * i:32 * i + 1, :],
                                              cl_sb[DH:DH + 1, :])
                    rbat = cxp.tile([128, 512], f32r, tag="rbat", bufs=1)
                    with nc.allow_low_precision(reason="f32r == f32 storage"):
                        nc.vector.reciprocal(out=rbat, in_=lb)
                    for i, (cl_sb, tb_idx) in enumerate(group):
                        rbp = psum.tile([128, 512], f32, tag="acc", bufs=2)
                        nc.tensor.matmul(rbp[0:DH, :], ones_sb[32 * i:32 * i + 1, :],
                                         rbat[32 * i:32 * i + 1, :],
                                         start=True, stop=True,
                                         tile_position=(32 * i, 0))
                        cx = cxp.tile([DH, 512], f32r, tag="cx")
                        nc.vector.tensor_mul(cx, cl_sb[0:DH, :], rbp[0:DH, :])
                        if SINGLE_A2A:
                            nc.sync.dma_start(
                                out=a2a_in_c[tb_idx, p0:p0 + DH, :], in_=cx)
                        else:
                            nc.sync.dma_start(out=a2a_in[h][tb_idx, :, :], in_=cx)
                    group.clear()

                for qi in range(4):
                    for b in range(B):
                        nkt = 4 * qi + 4       # causal kt tiles (128 wide)
                        q_lo = 2048 * b + 512 * qi
                        ets = []
                        for ch in range(nkt // 2):
                            stp = psum.tile([128, 2, 512], f32, tag="mm", bufs=3)
                            for j in range(2):
                                ktj = 2 * ch + j
                                k_lo = 2048 * b + 128 * ktj
                                nc.tensor.matmul(
                                    stp[:, j, :],
                                    kt_sb[p0:p0 + DH, k_lo:k_lo + 128],
                                    qt_sb[p0:p0 + DH, q_lo:q_lo + 512],
                                    start=True, stop=True,
                                )
                            if ch >= 2 * qi:  # diagonal chunk -> causal bias
                                moff = (ch - 2 * qi) * 2
                                nc.vector.tensor_add(stp, stp,
                                                     masks_sb[:, moff:moff + 2, :])
                            et = etp.tile([128, 2, 512], f32r, tag="et")
                            nc.scalar.activation(out=et, in_=stp, func=Exp,
                                                 scale=0.125)
                            ets.append(et)
                        # ctx^T (+ row sums via ones column): [65, 512]
                        ctxp = psum.tile([128, 512], f32, tag="acc", bufs=2)
                        for ch in range(nkt // 2):
                            for j in range(2):
                                ktj = 2 * ch + j
                                t_idx = 16 * b + ktj
                                nc.tensor.matmul(
                                    ctxp[0:DH + 1, :],
                                    v5_sb[:, t_idx, 65 * h:65 * h + 65],
                                    ets[ch][:, j, :],
                                    start=(ktj == 0), stop=(ktj == nkt - 1),
                                )
                        # copy ctx+l out of PSUM eagerly (PSUM slot recycles)
                        cl_sb = cxp.tile([DH + 1, 512], f32, tag="cl", bufs=5)
                        nc.vector.tensor_copy(cl_sb, ctxp[0:DH + 1, :])
                        group.append((cl_sb, 4 * b + qi))
                        if len(group) == 4:
                            flush_group()
                assert not group
                if not SINGLE_A2A:
                    # per-head exchange: h=0 overlaps h=1 compute
                    nc.gpsimd.collective_compute(
                        "AllToAll", mybir.AluOpType.bypass,
                        replica_groups=[list(range(N_CORES))],
                        ins=[a2a_in[h][:].opt()], outs=[a2a_out[h][:].opt()],
                    )
            if SINGLE_A2A:
                nc.gpsimd.collective_compute(
                    "AllToAll", mybir.AluOpType.bypass,
                    replica_groups=[list(range(N_CORES))],
                    ins=[a2a_in_c[:].opt()], outs=[a2a_out_c[:].opt()],
                )

            # ---- phase 3: output projection, per-half accumulation ----
            wo_eng = nc.sync if NO_SCALAR_DMA else nc.scalar
            wo_eng.dma_start(out=wo_sb,
                             in_=wo.ap().rearrange("(k p) o -> p k o", p=128))
            wo_eng.dma_start(out=bo_sb,
                             in_=bo.ap().rearrange("(k p) one -> p k one", p=128))
            cxt0s, cxt1s = [], []
            for kc in range(8):
                cxt0 = outp.tile([128, 512], f32r, tag="cxt0", bufs=8)
                cxt1 = outp.tile([128, 512], f32r, tag="cxt1", bufs=8)
                if SINGLE_A2A:
                    nc.sync.dma_start(out=cxt0[0:DH, :], in_=a2a_out_c[kc, 0:DH, :])
                    nc.scalar.dma_start(out=cxt1[DH:128, :], in_=a2a_out_c[kc, DH:128, :])
                else:
                    nc.sync.dma_start(out=cxt0[0:DH, :], in_=a2a_out[0][kc, :, :])
                    nc.scalar.dma_start(out=cxt1[DH:128, :], in_=a2a_out[1][kc, :, :])
                cxt0s.append(cxt0)
                cxt1s.append(cxt1)
            s0s = []
            for oc in range(8):  # h=0 half: runs as soon as A2A#1 lands
                op0 = psum.tile([128, 512], f32, tag="acc", bufs=2)
                osl = slice(128 * oc, 128 * (oc + 1))
                for kc in range(8):
                    nc.tensor.matmul(op0, wo_sb[0:DH, kc, osl], cxt0s[kc][0:DH, :],
                                     start=(kc == 0), stop=(kc == 7))
                s0 = outp.tile([128, 512], f32, tag="s0", bufs=8)
                nc.vector.tensor_copy(s0, op0)
                s0s.append(s0)
            for oc in range(8):  # h=1 half after A2A#2, then combine
                op1 = psum.tile([128, 512], f32, tag="acc", bufs=2)
                osl = slice(128 * oc, 128 * (oc + 1))
                for kc in range(8):
                    nc.tensor.matmul(op1, wo_sb[DH:128, kc, osl],
                                     cxt1s[kc][DH:128, :],
                                     start=(kc == 0), stop=(kc == 7))
                s1 = outp.tile([128, 512], f32, tag="s1", bufs=2)
                nc.vector.tensor_add(s1, s0s[oc], op1)
                osb = outp.tile([128, 512], f32, tag="osb", bufs=2)
                nc.scalar.activation(out=osb, in_=s1, func=Tanh, bias=bo_sb[:, oc, :])
                nc.sync.dma_start(out=out[osl, :], in_=osb)

    nc.compile()
    return nc


def _get_nc():
    if "nc" not in _CACHE:
        _CACHE["nc"] = _build()
    return _CACHE["nc"]


def _make_masks():
    kt_local = np.arange(128)[:, None, None]
    j = np.arange(4)[None, :, None]
    q_local = np.arange(512)[None, None, :]
    import ml_dtypes
    return np.where(q_local >= 128 * j + kt_local, 0.0, NEG).astype(ml_dtypes.bfloat16)


def kernel(states, Wq, bq, Wk, bk, Wv, bv, Wo, bo):
    global LAST_RESULTS
    states = np.asarray(states, dtype=np.float32)
    Wq, Wk, Wv, Wo = (np.asarray(w, dtype=np.float32) for w in (Wq, Wk, Wv, Wo))
    bq, bk, bv, bo = (np.asarray(x, dtype=np.float32) for x in (bq, bk, bv, bo))

    import ml_dtypes
    statesT = np.ascontiguousarray(states.reshape(T, D).T).astype(ml_dtypes.bfloat16)
    masks = _make_masks()
    ident = np.eye(128, dtype=np.float32)
    ones = np.ones((128, 64), dtype=np.float32)

    in_maps = []
    for c in range(N_CORES):
        sl = slice(LC * c, LC * (c + 1))
        in_maps.append({
            "statesT": statesT,
            "wq": np.ascontiguousarray(Wq[:, sl]).astype(ml_dtypes.bfloat16),
            "wk": np.ascontiguousarray(Wk[:, sl]).astype(ml_dtypes.bfloat16),
            "wv": np.ascontiguousarray(Wv[:, sl]).astype(ml_dtypes.bfloat16),
            "wo": Wo,
            "bq": np.ascontiguousarray(bq[sl]).reshape(LC, 1),
            "bk": np.ascontiguousarray(bk[sl]).reshape(LC, 1),
            "bv": np.ascontiguousarray(bv[sl]).reshape(LC, 1),
            "bo": bo.reshape(D, 1),
            "masks": masks,
            "ident": ident,
            "ones": ones,
        })

    nc = _get_nc()
    res = run_bass_kernel_spmd(nc, in_maps, core_ids=list(range(N_CORES)))
    LAST_RESULTS = res

    full = np.empty((T, D), dtype=np.float32)
    for c in range(N_CORES):
        full[TBLK * c:TBLK * (c + 1), :] = res.results[c]["out"].T
    return full.reshape(B, S, D)
